# revision 7
# baseline (speedup 1.0000x reference)
"""Trainium2 Bass kernel for nn_ChunkedQuantHead.

Computation (see reference):
  xc   = x.reshape(B, 16, 256)
  acts = mean(|xc|, axis=(0, 2))           # global per-chunk stat
  top4 = top_k(acts, 4)                    # global chunk routing
  routed = einsum('bkc,koc->bo', xc[:, top4], expert_w[top4]) + expert_b
  w_eff  = quant_w if max(acts) > 0.5 else sign(quant_w)*mean|quant_w|
  out    = routed @ w_eff.T + quant_b

Strategy (8 cores, data-parallel over batch):
  - The quantized head is FOLDED into the expert weights on the host:
    w2[c, f, v, j] = sum_o expert_w[c, o, f] * weff_v[j, o] for both
    weight variants v (quant / binary).  The kernel computes
    y2[b, c, v, j] = sum_f x[b, c*256+f] w2[c, f, v, j] in a single
    streaming pass, and the tail is a tiny masked combine:
    out[b, j] = sum_c mask_c * (cond ? y2[..q..] : y2[..bin..]) + bias.
  - Each core streams its 2048x4096 f32 shard from HBM ONCE (bf16 cast
    during the SWDGE DMA).  Per tile: DVE computes per-chunk |x| sums
    (fp16 partials), PE transposes all 32 [128,128] blocks (scalar
    engine copies PSUM->SBUF), and 32 accumulated matmuls (N=20)
    produce y2 (stored bf16).
  - Per-core chunk stats are exchanged with two small AllReduces
    (tiles 0..5 early / 6..15 late).  The last tile's DMA+stats are
    split in half and its partition-combine rides an accumulated PSUM
    matmul so only ~4.5us of stats work trails the DMA stream.
  - Top-4 selection is done with masks (sum of top-4 needs no
    ordering), so there is no dynamic control flow.
"""

import numpy as np

import concourse.bacc as bacc
import concourse.tile as tile
import concourse.mybir as mybir
from concourse.bass_utils import run_bass_kernel_spmd

F32 = mybir.dt.float32
F16 = mybir.dt.float16
BF16 = mybir.dt.bfloat16
AX = mybir.AxisListType
OP = mybir.AluOpType

N_CORES = 8
B, F = 16384, 4096
CHUNKS, CHUNK, OUT = 16, 256, 10
TOPK = 4
THRESH = 0.5
NV = 2                       # weight variants: quant / binary
W2 = NV * OUT                # 20 cols per chunk in the folded weights
BC = B // N_CORES            # 2048 rows per core
P = 128
TILES = BC // P              # 16 tiles of 128 rows
SUM_THRESH = THRESH * B * CHUNK  # compare sum(|x|) against this (scale folded)
BIG_NEG = -1.0e30
GT = 4                       # tiles per tail combine group
AG_SPLIT = 6                 # early AllReduce covers tiles 0..5
CW = CHUNKS * W2             # 320 coef columns

_CACHE = {}


def _build():
    nc = bacc.Bacc(
        "TRN2",
        target_bir_lowering=False,
        debug=False,
        num_devices=N_CORES,
    )

    x_d = nc.dram_tensor("x", [BC, F], F32, kind="ExternalInput")
    idb_d = nc.dram_tensor("id_bf", [P, P], BF16, kind="ExternalInput")
    # folded weights, pre-arranged host-side:
    #   w2_sb[p, h*CHUNKS*W2 + c*W2 + v*OUT + j]
    #     = sum_o expert_w[c, o, h*128+p] * weff_v[j, o]      (bf16)
    w_d = nc.dram_tensor("w2_sb", [P, 2 * CW], BF16, kind="ExternalInput")
    # folded biases: row [1, 2*OUT] = [bias_delta(=bias_q-bias_b) | bias_b]
    bias_d = nc.dram_tensor("bias2", [1, 2 * OUT], F32, kind="ExternalInput")
    out_d = nc.dram_tensor("out", [BC, OUT], F32, kind="ExternalOutput")

    with tile.TileContext(nc) as tc:
        with (
            tc.tile_pool(name="const", bufs=1) as constp,
            tc.tile_pool(name="persist", bufs=1) as perp,
            tc.tile_pool(name="xb", bufs=6) as xbp,
            tc.tile_pool(name="xt", bufs=3) as xtp,
            tc.tile_pool(name="tail", bufs=2) as tailp,
            tc.tile_pool(name="ps_misc", bufs=2, space="PSUM") as psm,
            tc.tile_pool(name="dram", bufs=1, space="DRAM") as dramp,
        ):
            # ---- persistent accumulators ----
            y2_all = perp.tile([P, TILES * CW], BF16)           # [128, 5120]
            red_all = perp.tile([P, TILES * CHUNKS], F16)       # [128, 256]

            # ---- constants ----
            id_bf = constp.tile([P, P], BF16)
            nc.sync.dma_start(id_bf[:, :], idb_d.ap())
            w2_sb = constp.tile([P, 2 * CW], BF16)
            nc.sync.dma_start(w2_sb[:, :], w_d.ap())
            bias2 = constp.tile([1, 2 * OUT], F32)
            nc.sync.dma_start(bias2[:, :], bias_d.ap())
            ones_col = constp.tile([P, 1], F32)
            nc.vector.memset(ones_col[:, :], 1.0)
            ones_col16 = constp.tile([P, 1], F16)
            nc.vector.memset(ones_col16[:, :], 1.0)
            ones_row = constp.tile([1, P], F32)
            nc.vector.memset(ones_row[:, :], 1.0)

            # DRAM bounce buffers for the two AllReduces
            cc1_in = dramp.tile([1, CHUNKS], F32)
            cc1_out = dramp.tile([1, CHUNKS], F32)
            cc2_in = dramp.tile([1, CHUNKS], F32)
            cc2_out = dramp.tile([1, CHUNKS], F32)

            def emit_allreduce(cc_in, cc_out):
                nc.gpsimd.collective_compute(
                    "AllReduce",
                    OP.add,
                    replica_groups=[list(range(N_CORES))],
                    ins=[cc_in.opt()],
                    outs=[cc_out.opt()],
                )

            # ---- main pass over x: stats + folded all-chunk projection ----
            with (
                tc.tile_pool(name="ps_tr", bufs=2, space="PSUM") as pstr,
                tc.tile_pool(name="ps_y", bufs=2, space="PSUM") as psy,
            ):
                ps_a2 = None
                for t in range(TILES):
                    xb = xbp.tile([P, F], BF16, tag="xb")
                    # SWDGE DMA with f32 -> bf16 cast in the datapath.
                    # The last tile is split in halves so its stats can
                    # start while the second half is still in flight.
                    if t == TILES - 1:
                        HF = F // 2
                        for hh in range(2):
                            nc.gpsimd.dma_start(
                                xb[:, hh * HF:(hh + 1) * HF],
                                x_d.ap()[t * P:(t + 1) * P, hh * HF:(hh + 1) * HF],
                            )
                            with nc.allow_low_precision(reason="fp16 partials"):
                                nc.vector.tensor_reduce(
                                    red_all[:, t * CHUNKS + hh * (CHUNKS // 2):
                                            t * CHUNKS + (hh + 1) * (CHUNKS // 2)],
                                    xb[:, hh * HF:(hh + 1) * HF].rearrange(
                                        "p (c f) -> p c f", f=CHUNK
                                    ),
                                    axis=AX.X,
                                    op=OP.add,
                                    apply_absolute_value=True,
                                )
                        # finish the partition-combine for tiles 6..15:
                        # accumulate tile 15's raw fp16 partials on top of
                        # the pre-reduced tiles 6..14 already in PSUM
                        nc.tensor.matmul(
                            ps_a2[:, :], lhsT=ones_col16[:, :],
                            rhs=red_all[:, t * CHUNKS:(t + 1) * CHUNKS],
                            start=False, stop=True,
                        )
                        cc_sb2 = tailp.tile([1, CHUNKS], F32, tag="cc_sb2")
                        nc.scalar.copy(cc_sb2[:, :], ps_a2[:, :])
                        nc.sync.dma_start(cc2_in[:, :], cc_sb2[:, :])
                        emit_allreduce(cc2_in, cc2_out)
                    else:
                        nc.gpsimd.dma_start(
                            xb[:, :], x_d.ap()[t * P:(t + 1) * P, :]
                        )
                        # per-chunk sum of |x| (fused abs+reduce, fp16 dst)
                        with nc.allow_low_precision(reason="fp16 partials"):
                            nc.vector.tensor_reduce(
                                red_all[:, t * CHUNKS:(t + 1) * CHUNKS],
                                xb[:, :].rearrange("p (c f) -> p c f", f=CHUNK),
                                axis=AX.X,
                                op=OP.add,
                                apply_absolute_value=True,
                            )

                    if t == TILES - 2:
                        # pre-reduce tiles 6..14 and park the partial in
                        # PSUM (start=True) while tile 15 is in flight
                        acts_b = tailp.tile([P, CHUNKS], F16, tag="acts_b")
                        with nc.allow_low_precision(reason="fp16 partials"):
                            nc.vector.tensor_reduce(
                                acts_b[:, :],
                                red_all[:, AG_SPLIT * CHUNKS:(TILES - 1) * CHUNKS]
                                .rearrange("p (t c) -> p c t", c=CHUNKS),
                                axis=AX.X,
                                op=OP.add,
                            )
                        ps_a2 = psm.tile([1, CHUNKS], F32, tag="psmisc")
                        nc.tensor.matmul(
                            ps_a2[:, :], lhsT=ones_col16[:, :], rhs=acts_b[:, :],
                            start=True, stop=False,
                        )

                    # transpose all 32 [128,128] blocks: x[b, f] -> xT[f, b]
                    xt = xtp.tile([P, F], BF16, tag="xt")
                    for g in range(2):
                        ps = pstr.tile([P, 16 * P], BF16, tag="ps_tr")
                        for j in range(16):
                            k = 16 * g + j
                            nc.tensor.transpose(
                                ps[:, j * P:(j + 1) * P],
                                xb[:, k * P:(k + 1) * P],
                                id_bf[:, :],
                            )
                        nc.scalar.copy(
                            xt[:, g * 16 * P:(g + 1) * 16 * P], ps[:, :]
                        )

                    # project every chunk through both folded weight variants
                    psy_t = psy.tile([P, CW], F32, tag="psy")
                    for c in range(CHUNKS):
                        for h in range(2):
                            kh = 2 * c + h
                            nc.tensor.matmul(
                                psy_t[:, c * W2:(c + 1) * W2],
                                lhsT=xt[:, kh * P:(kh + 1) * P],
                                rhs=w2_sb[:, h * CW + c * W2:
                                          h * CW + (c + 1) * W2],
                                start=(c == 0 and h == 0),
                                stop=(c == CHUNKS - 1 and h == 1),
                            )
                    nc.scalar.copy(
                        y2_all[:, t * CW:(t + 1) * CW],
                        psy_t[:, :],
                    )

                    if t == AG_SPLIT - 1:
                        # early AllReduce covering tiles 0..AG_SPLIT-1 --
                        # overlaps with the rest of the main loop
                        acts_a = tailp.tile([P, CHUNKS], F32, tag="acts_a")
                        nc.vector.tensor_reduce(
                            acts_a[:, :],
                            red_all[:, 0:AG_SPLIT * CHUNKS].rearrange(
                                "p (t c) -> p c t", c=CHUNKS
                            ),
                            axis=AX.X,
                            op=OP.add,
                        )
                        ps_a1 = psm.tile([1, CHUNKS], F32, tag="psmisc")
                        nc.tensor.matmul(
                            ps_a1[:, :], lhsT=ones_col[:, :], rhs=acts_a[:, :],
                            start=True, stop=True,
                        )
                        cc_sb1 = tailp.tile([1, CHUNKS], F32, tag="cc_sb1")
                        nc.scalar.copy(cc_sb1[:, :], ps_a1[:, :])
                        nc.sync.dma_start(cc1_in[:, :], cc_sb1[:, :])
                        emit_allreduce(cc1_in, cc1_out)

            # ---- S = AR1 + AR2 (each already summed over cores) ----
            Sg2 = tailp.tile([1, 2 * CHUNKS], F32, tag="Sg2")
            nc.sync.dma_start(Sg2[:, 0:CHUNKS], cc1_out[:, :])
            nc.sync.dma_start(Sg2[:, CHUNKS:2 * CHUNKS], cc2_out[:, :])
            S = tailp.tile([1, CHUNKS], F32, tag="S")
            nc.vector.tensor_reduce(
                S[:, :],
                Sg2[:, :].rearrange("p (a c) -> p c a", a=2),
                axis=AX.X,
                op=OP.add,
            )

            # ---- top-4 threshold via 4x (max + mask-out); all on partition 0
            cur = tailp.tile([1, CHUNKS], F32, tag="cur")
            nc.vector.tensor_copy(cur[:, :], S[:, :])
            m1 = None
            mk = None
            for k in range(TOPK):
                mk = tailp.tile([1, 1], F32, tag=f"mk{k}")
                nc.vector.tensor_reduce(mk[:, :], cur[:, :], axis=AX.X, op=OP.max)
                if k == 0:
                    m1 = mk
                if k < TOPK - 1:
                    sel = tailp.tile([1, CHUNKS], F32, tag="sel")
                    # sel = (cur >= mk) * BIG_NEG  in one fused op
                    nc.vector.tensor_scalar(
                        sel[:, :], cur[:, :], mk[:, :], BIG_NEG,
                        op0=OP.is_ge, op1=OP.mult,
                    )
                    nc.vector.tensor_tensor(cur[:, :], cur[:, :], sel[:, :], op=OP.add)
            m4 = mk  # 4th largest

            mask16 = tailp.tile([1, CHUNKS], F32, tag="mask16")
            nc.vector.tensor_scalar(
                mask16[:, :], S[:, :], m4[:, :], None, op0=OP.is_ge
            )
            cond = tailp.tile([1, 1], F32, tag="cond")
            nc.vector.tensor_scalar(
                cond[:, :], m1[:, :], float(SUM_THRESH), None, op0=OP.is_gt
            )

            # ---- coefficient row: [0:320] mask_c * variant-select,
            #      [320:330] cond-selected bias ----
            BROW = CW + OUT
            selpair = tailp.tile([1, W2], F32, tag="selpair")
            nc.vector.tensor_scalar(
                selpair[:, 0:OUT], ones_row[:, 0:OUT], cond[:, :], None,
                op0=OP.mult,
            )
            nc.vector.tensor_scalar(
                selpair[:, OUT:W2], ones_row[:, 0:OUT], cond[:, :], None,
                op0=OP.subtract,
            )
            brow = tailp.tile([1, BROW], F32, tag="brow")
            # brow[(c, w)] = mask16[c] * selpair[w]  via stride-0 broadcasts
            nc.vector.tensor_tensor(
                brow[:, 0:CW].rearrange("p (c w) -> p c w", w=W2),
                mask16[:, :, None].broadcast_to([1, CHUNKS, W2]),
                selpair[:, None, :].broadcast_to([1, CHUNKS, W2]),
                op=OP.mult,
            )
            # bias_sel = bias_b + cond * (bias_q - bias_b)
            nc.vector.tensor_scalar(
                brow[:, CW:BROW], bias2[:, 0:OUT], cond[:, :], None,
                op0=OP.mult,
            )
            nc.vector.tensor_tensor(
                brow[:, CW:BROW], brow[:, CW:BROW],
                bias2[:, OUT:2 * OUT], op=OP.add,
            )

            # ---- broadcast row -> all partitions via K=1 matmul ----
            ps_b = psm.tile([P, BROW], F32, tag="psmisc")
            nc.tensor.matmul(
                ps_b[:, :], lhsT=ones_row[:, :], rhs=brow[:, :],
                start=True, stop=True,
            )
            bc = tailp.tile([P, BROW], BF16, tag="bc")
            nc.scalar.copy(bc[:, :], ps_b[:, :])

            # ---- masked combine: 4 tiles per group, all on DVE ----
            out_stage = perp.tile([P, TILES * OUT], F32)
            for g in range(TILES // GT):
                t0 = g * GT
                tmp = tailp.tile([P, GT * CW], BF16, tag="tmpg")
                nc.vector.tensor_tensor(
                    tmp[:, :].rearrange("p (u w) -> p u w", w=CW),
                    y2_all[:, t0 * CW:(t0 + GT) * CW].rearrange(
                        "p (u w) -> p u w", w=CW
                    ),
                    bc[:, None, 0:CW].broadcast_to([P, GT, CW]),
                    op=OP.mult,
                )
                # reduce over (c, v): [p, (u c v j)] -> [p, (u j)]
                nc.vector.tensor_reduce(
                    out_stage[:, t0 * OUT:(t0 + GT) * OUT].rearrange(
                        "p (u j) -> p u j", j=OUT
                    ),
                    tmp[:, :].rearrange(
                        "p (u c v j) -> p u j (c v)", u=GT, v=NV, j=OUT
                    ),
                    axis=AX.X,
                    op=OP.add,
                )
                nc.vector.tensor_tensor(
                    out_stage[:, t0 * OUT:(t0 + GT) * OUT].rearrange(
                        "p (u j) -> p u j", j=OUT
                    ),
                    out_stage[:, t0 * OUT:(t0 + GT) * OUT].rearrange(
                        "p (u j) -> p u j", j=OUT
                    ),
                    bc[:, None, CW:BROW].broadcast_to([P, GT, OUT]),
                    op=OP.add,
                )
                # one DMA per group: SBUF [p, (u j)] -> DRAM rows (t0+u)*128+p
                nc.sync.dma_start(
                    out_d.ap()[t0 * P:(t0 + GT) * P, :].rearrange(
                        "(u p) o -> p u o", p=P
                    ),
                    out_stage[:, t0 * OUT:(t0 + GT) * OUT].rearrange(
                        "p (u o) -> p u o", o=OUT
                    ),
                )

    nc.compile()
    return nc


def _get_nc():
    if "nc" not in _CACHE:
        _CACHE["nc"] = _build()
    return _CACHE["nc"]


def _host_prep(expert_w, expert_b, quant_w, quant_b):
    """Fold the quantized head into the expert weights (tiny tensors)."""
    import ml_dtypes

    qmean = np.float32(np.mean(np.abs(quant_w)))
    weff_q = quant_w.astype(np.float32)
    weff_b = (np.sign(quant_w) * qmean).astype(np.float32)
    # w2[c, f, v, j] = sum_o expert_w[c, o, f] * weff_v[j, o]
    w2 = np.stack(
        [
            np.einsum("cof,jo->cfj", expert_w, weff_q),
            np.einsum("cof,jo->cfj", expert_w, weff_b),
        ],
        axis=2,
    )  # [CHUNKS, CHUNK, NV, OUT]
    # w2_sb[p, h*CHUNKS*W2 + c*W2 + v*OUT + j] = w2[c, h*128+p, v, j]
    wr = w2.reshape(CHUNKS, 2, P, NV, OUT)          # c, h, p, v, j
    w2_sb = np.ascontiguousarray(
        wr.transpose(2, 1, 0, 3, 4).reshape(P, 2 * CHUNKS * W2)
    ).astype(ml_dtypes.bfloat16)
    bias_q = weff_q @ expert_b + quant_b
    bias_b = weff_b @ expert_b + quant_b
    bias2 = np.concatenate([bias_q - bias_b, bias_b]).reshape(1, 2 * OUT)
    return w2_sb, np.ascontiguousarray(bias2.astype(np.float32))


def kernel(x, expert_w, expert_b, quant_w, quant_b):
    import ml_dtypes

    x = np.ascontiguousarray(np.asarray(x, dtype=np.float32))
    expert_w = np.asarray(expert_w, dtype=np.float32)
    expert_b = np.asarray(expert_b, dtype=np.float32)
    quant_w = np.asarray(quant_w, dtype=np.float32)
    quant_b = np.asarray(quant_b, dtype=np.float32)

    w2_sb, bias2 = _host_prep(expert_w, expert_b, quant_w, quant_b)
    id_bf = np.eye(P, dtype=ml_dtypes.bfloat16)

    nc = _get_nc()
    in_maps = []
    for i in range(N_CORES):
        in_maps.append({
            "x": np.ascontiguousarray(x[i * BC:(i + 1) * BC]),
            "w2_sb": w2_sb,
            "bias2": bias2,
            "id_bf": id_bf,
        })

    res = run_bass_kernel_spmd(nc, in_maps, core_ids=list(range(N_CORES)))
    out = np.concatenate(
        [np.asarray(res.results[i]["out"]) for i in range(N_CORES)], axis=0
    )
    return out.astype(np.float32)
